# revision 3
# baseline (speedup 1.0000x reference)
"""GQA kernel for Trainium2, 8 NeuronCores.

Sharding: core c -> (batch b = c//4, head-group g = c%4).
Each core owns 4 query heads (g*4..g*4+3) and their shared KV head g, full
sequence. Host pre-transposes activations (x^T so the contraction dim d lands
on partitions), device computes Q/K/V projections, scores in both
orientations (natural [i,j] for the attn_weight output + softmax stats via
ACT accum; transposed [j,i] for the O = P@V matmuls), normalizes with
per-partition scalars, PE-transposes O for the row-parallel Wo projection,
and the host sums the 4 partial outputs per batch and adds bo.

All matmuls run in float32r (TF32-class, full PE rate).
"""

import sys

sys.path.insert(0, "/opt/trn_rl_repo")

from contextlib import ExitStack

import numpy as np

import concourse.bass as bass
import concourse.tile as tile
from concourse import bacc, mybir
from concourse.masks import make_identity

B, T, D = 2, 2048, 2048
H, KH = 16, 4
DK = 128
NH = H // KH  # heads per core = 4
GD = NH * DK  # 512, per-core q/o head dim
P = 128
F32 = mybir.dt.float32
F32R = mybir.dt.float32r


def build(d=D, t=T):
    dt_n = d // P   # d tiles (contraction)
    it_n = t // P   # query tiles
    jt_n = t // P   # key tiles
    ih_n = 2        # i halves
    ihw = t // 2    # i half width
    scale = float(DK) ** -0.5

    nc = bacc.Bacc()
    xq = nc.declare_dram_parameter("xq", [d, t], F32R, isOutput=False)  # q[b].T
    xk = nc.declare_dram_parameter("xk", [d, t], F32R, isOutput=False)
    xv = nc.declare_dram_parameter("xv", [d, t], F32R, isOutput=False)
    wq = nc.declare_dram_parameter("wq", [d, GD], F32R, isOutput=False)
    wk = nc.declare_dram_parameter("wk", [d, DK], F32R, isOutput=False)
    wv = nc.declare_dram_parameter("wv", [d, DK], F32R, isOutput=False)
    wo = nc.declare_dram_parameter("wo", [GD, d], F32R, isOutput=False)
    bq = nc.declare_dram_parameter("bq", [P, NH], F32, isOutput=False)
    bk = nc.declare_dram_parameter("bk", [P, 1], F32, isOutput=False)
    bv = nc.declare_dram_parameter("bv", [P, 1], F32, isOutput=False)
    attn = nc.declare_dram_parameter("attn", [NH, t, t], F32, isOutput=True)
    outp = nc.declare_dram_parameter("outp", [t, d], F32, isOutput=True)

    xq_r = xq.ap().rearrange("(dt p) t -> dt p t", p=P)
    xk_r = xk.ap().rearrange("(dt p) t -> dt p t", p=P)
    xv_r = xv.ap().rearrange("(dt p) t -> dt p t", p=P)
    wq_r = wq.ap().rearrange("(dt p) m -> p dt m", p=P)
    wk_r = wk.ap().rearrange("(dt p) m -> p dt m", p=P)
    wv_r = wv.ap().rearrange("(dt p) m -> p dt m", p=P)
    wo_r = wo.ap().rearrange("(hh p) e -> p hh e", p=P)
    attn_r = attn.ap().rearrange("h (it p) j -> h it p j", p=P)
    outp_r = outp.ap().rearrange("(it p) e -> it p e", p=P)

    with ExitStack() as ctx:
        tc = ctx.enter_context(tile.TileContext(nc))

        const = ctx.enter_context(tc.tile_pool(name="const", bufs=1))
        ident = const.tile([P, P], F32)
        make_identity(nc, ident)
        bq_sb = const.tile([P, NH], F32)
        bk_sb = const.tile([P, 1], F32)
        bv_sb = const.tile([P, 1], F32)
        nc.sync.dma_start(out=bq_sb, in_=bq.ap())
        nc.sync.dma_start(out=bk_sb, in_=bk.ap())
        nc.sync.dma_start(out=bv_sb, in_=bv.ap())
        wk_sb = const.tile([P, dt_n, DK], F32R)
        wv_sb = const.tile([P, dt_n, DK], F32R)
        nc.sync.dma_start(out=wk_sb, in_=wk_r)
        nc.sync.dma_start(out=wv_sb, in_=wv_r)
        wo_sb = const.tile([P, NH, d], F32R)
        nc.sync.dma_start(out=wo_sb, in_=wo_r)

        # persistent per-core intermediates
        kt_sb = const.tile([P, t], F32R)         # K^T [dk, t]
        v_sb = const.tile([P, jt_n, DK], F32R)   # V   [t-tile, dv]
        qt_sb = const.tile([P, NH, t], F32R)     # Q^T [dk, (h, i)]
        ot_sb = const.tile([P, NH, t], F32R)     # O^T [dv, (h, i)] normalized
        r_sb = const.tile([P, NH, it_n], F32)    # softmax 1/den per (h, i)

        nch = t // 512  # 512-wide column chunks per row of t

        # ---- Phase A: K^T and V^T projections, then V = transpose(V^T)
        with tc.tile_pool(name="kv_ps", bufs=1, space="PSUM") as kv_ps, \
             tc.tile_pool(name="xkv", bufs=2) as xkv:
            ktp = kv_ps.tile([P, t], F32)
            vtp = kv_ps.tile([P, t], F32)
            for dti in range(dt_n):
                xk_t = xkv.tile([P, t], F32R, tag="xk")
                xv_t = xkv.tile([P, t], F32R, tag="xv")
                nc.sync.dma_start(out=xk_t, in_=xk_r[dti])
                nc.sync.dma_start(out=xv_t, in_=xv_r[dti])
                st = dti == 0
                sp = dti == dt_n - 1
                for c in range(nch):
                    cs = slice(c * 512, (c + 1) * 512)
                    nc.tensor.matmul(ktp[:, cs], wk_sb[:, dti, :], xk_t[:, cs],
                                     start=st, stop=sp)
                    nc.tensor.matmul(vtp[:, cs], wv_sb[:, dti, :], xv_t[:, cs],
                                     start=st, stop=sp)
            vt_sb = const.tile([P, t], F32R)
            nc.vector.tensor_scalar_add(kt_sb, ktp, bk_sb)
            nc.vector.tensor_scalar_add(vt_sb, vtp, bv_sb)

        with tc.tile_pool(name="vtr_ps", bufs=2, space="PSUM") as vtr_ps:
            for tt in range(jt_n):
                vnp = vtr_ps.tile([P, P], F32)
                nc.tensor.transpose(
                    vnp, vt_sb.bitcast(F32)[:, tt * P:(tt + 1) * P], ident)
                nc.vector.tensor_copy(v_sb[:, tt, :], vnp)

        # ---- Phase B: Q^T projection (per i-half, 4 heads x half-width psum)
        with tc.tile_pool(name="q_ps", bufs=1, space="PSUM") as q_ps, \
             tc.tile_pool(name="xq_pool", bufs=2) as xq_pool, \
             tc.tile_pool(name="wq_pool", bufs=2) as wq_pool:
            for ih in range(ih_n):
                hs = slice(ih * ihw, (ih + 1) * ihw)
                qp = [q_ps.tile([P, ihw], F32, name=f"qp{ih}_{h}", tag=f"qp{h}")
                      for h in range(NH)]
                for dti in range(dt_n):
                    xq_t = xq_pool.tile([P, ihw], F32R, tag="xq")
                    nc.sync.dma_start(out=xq_t, in_=xq_r[dti][:, hs])
                    wq_t = wq_pool.tile([P, GD], F32R, tag="wq")
                    nc.sync.dma_start(out=wq_t, in_=wq_r[:, dti, :])
                    st = dti == 0
                    sp = dti == dt_n - 1
                    for h in range(NH):
                        for c in range(ihw // 512):
                            cs = slice(c * 512, (c + 1) * 512)
                            nc.tensor.matmul(
                                qp[h][:, cs],
                                wq_t[:, h * DK:(h + 1) * DK],
                                xq_t[:, cs], start=st, stop=sp)
                for h in range(NH):
                    nc.vector.tensor_scalar_add(
                        qt_sb[:, h, hs], qp[h], bq_sb[:, h:h + 1])

        # ---- Phase C: attention per head
        for h in range(NH):
            # C1: natural S -> P_norm -> attn out; softmax stats
            with tc.tile_pool(name=f"s_ps{h}", bufs=2, space="PSUM") as s_ps, \
                 tc.tile_pool(name=f"pn_pool{h}", bufs=3) as pn_pool, \
                 tc.tile_pool(name=f"den_pool{h}", bufs=4) as den_pool:
                for it in range(it_n):
                    sp_t = s_ps.tile([P, t], F32, tag="sp")
                    for c in range(nch):
                        cs = slice(c * 512, (c + 1) * 512)
                        nc.tensor.matmul(
                            sp_t[:, cs],
                            qt_sb[:, h, it * P:(it + 1) * P],
                            kt_sb[:, cs], start=True, stop=True)
                    pn = pn_pool.tile([P, t], F32, tag="pn")
                    den = den_pool.tile([P, 1], F32, tag="den")
                    nc.scalar.activation(
                        pn, sp_t, mybir.ActivationFunctionType.Exp,
                        scale=scale, accum_out=den)
                    nc.vector.reciprocal(r_sb[:, h, it:it + 1], den)
                    nc.vector.tensor_scalar_mul(pn, pn, r_sb[:, h, it:it + 1])
                    nc.sync.dma_start(out=attn_r[h, it], in_=pn)

            # C2: transposed S -> P^T (unnormalized) -> O accum; normalize +
            # transpose O into ot_sb
            with tc.tile_pool(name=f"st_ps{h}", bufs=2, space="PSUM") as st_ps, \
                 tc.tile_pool(name=f"o_ps{h}", bufs=1, space="PSUM") as o_ps, \
                 tc.tile_pool(name=f"ot_ps{h}", bufs=2, space="PSUM") as ot_ps, \
                 tc.tile_pool(name=f"pt_pool{h}", bufs=2) as pt_pool, \
                 tc.tile_pool(name=f"ob_pool{h}", bufs=4) as ob_pool:
                for ih in range(ih_n):
                    op_t = o_ps.tile([P, ihw], F32, name=f"op{h}_{ih}", tag="op")
                    nst = ihw // P  # i subtiles in this half
                    for jt in range(jt_n):
                        stp = st_ps.tile([P, ihw], F32, tag="stp")
                        for c in range(ihw // 512):
                            cs = slice(ih * ihw + c * 512, ih * ihw + (c + 1) * 512)
                            nc.tensor.matmul(
                                stp[:, c * 512:(c + 1) * 512],
                                kt_sb[:, jt * P:(jt + 1) * P],
                                qt_sb[:, h, cs], start=True, stop=True)
                        pt = pt_pool.tile([P, ihw], F32R, tag="pt")
                        nc.scalar.activation(
                            pt, stp, mybir.ActivationFunctionType.Exp,
                            scale=scale)
                        for sti in range(nst):
                            # psum zero-region: start only on the first write
                            # of each 2KB bank (4 subtiles/bank), stop on last
                            nc.tensor.matmul(
                                op_t[:, sti * P:(sti + 1) * P],
                                pt[:, sti * P:(sti + 1) * P],
                                v_sb[:, jt, :],
                                start=(jt == 0 and sti % 4 == 0),
                                stop=(jt == jt_n - 1 and sti % 4 == 3),
                                skip_group_check=True)
                    for sti in range(nst):
                        it = ih * nst + sti
                        o_sb = ob_pool.tile([P, DK], F32, tag="ob")
                        nc.vector.tensor_scalar_mul(
                            o_sb, op_t[:, sti * P:(sti + 1) * P],
                            r_sb[:, h, it:it + 1])
                        otp = ot_ps.tile([P, P], F32, tag="otp")
                        nc.tensor.transpose(otp, o_sb, ident)
                        nc.vector.tensor_copy(
                            ot_sb[:, h, it * P:(it + 1) * P], otp)

        # ---- Phase D: row-parallel output projection (partial sum over heads)
        with tc.tile_pool(name="u_ps", bufs=4, space="PSUM") as u_ps, \
             tc.tile_pool(name="ou_pool", bufs=2) as ou_pool:
            for it in range(it_n):
                ou = ou_pool.tile([P, d], F32, tag="ou")
                for ec in range(d // 512):
                    up = u_ps.tile([P, 512], F32, tag="up")
                    for hh in range(NH):
                        nc.tensor.matmul(
                            up,
                            ot_sb[:, hh, it * P:(it + 1) * P],
                            wo_sb[:, hh, ec * 512:(ec + 1) * 512],
                            start=(hh == 0), stop=(hh == NH - 1))
                    nc.vector.tensor_copy(ou[:, ec * 512:(ec + 1) * 512], up)
                nc.sync.dma_start(out=outp_r[it], in_=ou)

    nc.finalize()
    return nc


_built = {}


def _get_nc():
    if "nc" not in _built:
        _built["nc"] = build()
    return _built["nc"]


def _run(in_maps, **kw):
    from concourse.bass_utils import run_bass_kernel_spmd

    return run_bass_kernel_spmd(_get_nc(), in_maps, list(range(8)), **kw)


def _in_maps(q, k, v, Wq, bq, Wk, bk, Wv, bv, Wo, bo):
    q = np.asarray(q, np.float32)
    k = np.asarray(k, np.float32)
    v = np.asarray(v, np.float32)
    Wq = np.asarray(Wq, np.float32)
    Wk = np.asarray(Wk, np.float32)
    Wv = np.asarray(Wv, np.float32)
    Wo = np.asarray(Wo, np.float32)
    bq = np.asarray(bq, np.float32)
    bk = np.asarray(bk, np.float32)
    bv = np.asarray(bv, np.float32)
    bo = np.asarray(bo, np.float32)

    xqs = [np.ascontiguousarray(q[b].T) for b in range(B)]
    xks = [np.ascontiguousarray(k[b].T) for b in range(B)]
    xvs = [np.ascontiguousarray(v[b].T) for b in range(B)]
    q = np.asarray(q, np.float32)
    in_maps = []
    for c in range(8):
        b, g = c // 4, c % 4
        in_maps.append({
            "xq": xqs[b], "xk": xks[b], "xv": xvs[b],
            "wq": np.ascontiguousarray(Wq[:, g * GD:(g + 1) * GD]),
            "wk": np.ascontiguousarray(Wk[:, g * DK:(g + 1) * DK]),
            "wv": np.ascontiguousarray(Wv[:, g * DK:(g + 1) * DK]),
            "wo": np.ascontiguousarray(Wo[g * GD:(g + 1) * GD, :]),
            "bq": np.ascontiguousarray(bq[g * GD:(g + 1) * GD].reshape(NH, P).T),
            "bk": np.ascontiguousarray(bk[g * DK:(g + 1) * DK].reshape(P, 1)),
            "bv": np.ascontiguousarray(bv[g * DK:(g + 1) * DK].reshape(P, 1)),
        })

    return in_maps


def _assemble(res, bo):
    out = np.zeros((B, T, D), np.float32)
    attn = np.empty((B, H, T, T), np.float32)
    for c in range(8):
        b, g = c // 4, c % 4
        out[b] += res[c]["outp"]
        attn[b, g * NH:(g + 1) * NH] = res[c]["attn"]
    out += np.asarray(bo, np.float32)
    return out, attn


def kernel(q, k, v, Wq, bq, Wk, bk, Wv, bv, Wo, bo):
    in_maps = _in_maps(q, k, v, Wq, bq, Wk, bk, Wv, bv, Wo, bo)
    res = _run(in_maps).results
    return _assemble(res, bo)
